# revision 25
# baseline (speedup 1.0000x reference)
"""Chamfer distance kernel for Trainium2 (8 NeuronCores, data-parallel over batch).

Full inputs x, y: [8, 4096, 3] fp32. Output: [8] fp32.

Strategy (per core = one batch):
  dist(i,j) = ||x_i||^2 + ||y_j||^2 - 2 x_i.y_j  computed on the PE as a
  K=24 matmul using 3-way bf16 splits of coordinates and norms (exact to
  ~1e-5 abs, fp32-equivalent).  Distance tiles land in PSUM; the two min
  directions are computed with two matmul layouts (x-on-partitions and
  y-on-partitions).  Each [128, 4096] row-block is consumed by:
    - ScalarE copy of the odd 2-bank group PSUM->SBUF
    - one VectorE tensor_tensor_reduce: out=min(psum_grp, sbuf_grp),
      accum_out = min-reduce over free dim  -> per-point min column.
  Host sums the 128 per-partition row-sums and divides by n.
"""

import os
import sys

import numpy as np

for _p in ("/opt/trn_rl_repo", "/root/.axon_site/_ro/trn_rl_repo"):
    if os.path.isdir(_p) and _p not in sys.path:
        sys.path.insert(0, _p)

B = 8
N = 4096
D = 3
P = 128
IPP = N // P  # 32 points per partition
K = 24        # contraction rows
NCH = N // P  # 32 lhsT chunks of 128 points
HBANK = 1024  # free elems per 2-bank psum group
BIG = 3.0e38
MODE = os.environ.get("CHAMFER_MODE", "tts")  # "tts" | "reduce" | "ttr"

_CACHE = {}


def _build_nc():
    from contextlib import ExitStack

    from concourse import bacc, mybir
    from concourse.tile import TileContext

    f32 = mybir.dt.float32
    bf16 = mybir.dt.bfloat16
    MIN = mybir.AluOpType.min
    ADD = mybir.AluOpType.add
    AX = mybir.AxisListType.X

    nc = bacc.Bacc()
    x_d = nc.declare_dram_parameter("x", [N, D], f32, isOutput=False)
    y_d = nc.declare_dram_parameter("y", [N, D], f32, isOutput=False)
    res_d = nc.declare_dram_parameter("res", [P, 2], f32, isOutput=True)

    with ExitStack() as ctx:
        tc = ctx.enter_context(TileContext(nc))
        singles = ctx.enter_context(tc.tile_pool(name="singles", bufs=1))
        copies = ctx.enter_context(tc.tile_pool(name="copies", bufs=3))
        scratch = ctx.enter_context(tc.tile_pool(name="scratch", bufs=2))

        # ---------- operand prep: K-matrices KM[side] of shape [24, 4096] bf16
        # Row pairing (lhs row r multiplies rhs row r, summed over r):
        #   r0-2 : 2*xh_c * -yh_c     r3-5 : 2*xh_c * -ym_c
        #   r6-8 : 2*xm_c * -yh_c    r9-11: 2*xm_c * -ym_c
        #   r12-14: 2*xh_c * -yl_c    r15-17: 2*xl_c * -yh_c
        #   r18-20: nx{h,m,l} * 1     r21-23: 1 * ny{h,m,l}
        # Built by assembling W[side] = [128 pts, 32 blocks, 32 rows] bf16 and
        # PE-transposing each [128, 32] block into KM[24, block*128:...].
        from concourse import masks

        ident = singles.tile([P, P], bf16, tag="ident")
        masks.make_identity(nc, ident[:])

        KM = {}
        WS = {}
        for side, dram in (("y", y_d), ("x", x_d)):
            sc = 2.0 if side == "x" else -1.0
            raw = singles.tile([P, IPP, D], f32, tag=f"raw_{side}")
            nc.sync.dma_start(
                out=raw[:],
                in_=dram[:, :].rearrange("(p i) c -> p i c", p=P),
            )
            # 3-way bf16 split of coordinates (natural layout).
            # Casts/subs go to ScalarE/GpSimd to keep VectorE (the main-loop
            # bottleneck engine) free.
            h = singles.tile([P, IPP, D], bf16, tag=f"h_{side}")
            nc.scalar.copy(h[:], raw[:])
            e1 = singles.tile([P, IPP, D], f32, tag=f"e1_{side}")
            nc.gpsimd.tensor_sub(e1[:], raw[:], h[:])
            m = singles.tile([P, IPP, D], bf16, tag=f"m_{side}")
            nc.scalar.copy(m[:], e1[:])
            e2 = singles.tile([P, IPP, D], f32, tag=f"e2_{side}")
            nc.gpsimd.tensor_sub(e2[:], e1[:], m[:])
            low = singles.tile([P, IPP, D], bf16, tag=f"l_{side}")
            nc.scalar.copy(low[:], e2[:])
            # norms + 3-way split
            sq = singles.tile([P, IPP, D], f32, tag=f"sq_{side}")
            nc.gpsimd.tensor_mul(sq[:], raw[:], raw[:])
            nrm = singles.tile([P, IPP], f32, tag=f"nrm_{side}")
            nc.vector.tensor_reduce(nrm[:], sq[:], axis=AX, op=ADD)
            nh = singles.tile([P, IPP], bf16, tag=f"nh_{side}")
            nc.scalar.copy(nh[:], nrm[:])
            ne1 = singles.tile([P, IPP], f32, tag=f"ne1_{side}")
            nc.gpsimd.tensor_sub(ne1[:], nrm[:], nh[:])
            nm = singles.tile([P, IPP], bf16, tag=f"nm_{side}")
            nc.scalar.copy(nm[:], ne1[:])
            ne2 = singles.tile([P, IPP], f32, tag=f"ne2_{side}")
            nc.gpsimd.tensor_sub(ne2[:], ne1[:], nm[:])
            nl = singles.tile([P, IPP], bf16, tag=f"nl_{side}")
            nc.scalar.copy(nl[:], ne2[:])

            # staging tile W: [128 pts-in-block, 32 blocks, 32 rows] bf16
            w = singles.tile([P, IPP, 32], bf16, tag=f"w_{side}")
            nc.gpsimd.memset(w[:], 1.0)  # rows 21-23 / 18-20 stay ones; 24-31 pad
            if side == "x":
                rowsrc = [(h, sc), (h, sc), (m, sc), (m, sc), (h, sc), (low, sc)]
                norm0 = 18
            else:
                rowsrc = [(h, sc), (m, sc), (h, sc), (m, sc), (low, sc), (h, sc)]
                norm0 = 21
            for g, (arr, s) in enumerate(rowsrc):
                for c in range(D):
                    r = 3 * g + c
                    nc.vector.tensor_scalar_mul(w[:, :, r], arr[:, :, c], s)
            for c, arr in enumerate((nh, nm, nl)):
                nc.vector.tensor_copy(w[:, :, norm0 + c], arr[:])

            km = singles.tile([32, N], bf16, tag=f"km_{side}")
            KM[side] = km
            WS[side] = w

        # ---------- main: two layouts, each 32 chunks x 2 half-rows
        psum = ctx.enter_context(tc.tile_pool(name="psum", bufs=2, space="PSUM"))

        def emit_km_block(side, t4):
            # transpose 4 blocks into one PSUM bank, then copy out [32, 512]
            w = WS[side]
            km = KM[side]
            pt = psum.tile([32, 512], bf16, tag="p1")  # borrow p1 slots
            for u in range(4):
                t = t4 * 4 + u
                nc.tensor.transpose(
                    pt[:, u * P : (u + 1) * P], w[:, t, :], ident[:]
                )
            nc.scalar.copy(km[:, t4 * 512 : (t4 + 1) * 512], pt[:])

        # y-side KM needed in full before the first matmul; x-side KM blocks
        # are emitted just-in-time inside the layout-A chunk loop.
        for t4 in range(IPP // 4):
            emit_km_block("y", t4)
        rs_all = singles.tile([P, 2], f32, tag="rs_all")
        cols_per_chunk = {"tts": 1, "ttr": 2, "reduce": 4}[MODE]
        for li, (lhs_km, rhs_km) in enumerate(
            [(KM["x"], KM["y"]), (KM["y"], KM["x"])]
        ):
            acc = singles.tile([P, cols_per_chunk * NCH], f32, tag=f"acc_{li}")
            for c in range(NCH):
                if li == 0 and c % 4 == 0:
                    emit_km_block("x", c // 4)
                lhsT = lhs_km[0:K, c * P : (c + 1) * P]
                prev_junk = None
                for half in range(2):
                    p0 = psum.tile([P, HBANK], f32, tag="p0")
                    p1 = psum.tile([P, HBANK], f32, tag="p1")
                    for q in range(4):
                        j = half * 4 + q
                        dst = p0 if q < 2 else p1
                        col = (q % 2) * 512
                        nc.tensor.matmul(
                            dst[:, col : col + 512],
                            lhsT,
                            rhs_km[0:K, j * 512 : (j + 1) * 512],
                            start=True,
                            stop=True,
                        )
                    if MODE == "tts":
                        s1 = copies.tile([P, HBANK], f32, tag="s1")
                        nc.scalar.copy(s1[:], p1[:])
                        junk = scratch.tile([P, HBANK], f32, tag="junk")
                        nc.vector.tensor_tensor_scan(
                            out=junk[:],
                            data0=p0[:],
                            data1=s1[:],
                            initial=(
                                BIG if prev_junk is None
                                else prev_junk[:, HBANK - 1 : HBANK]
                            ),
                            op0=MIN,
                            op1=MIN,
                        )
                        prev_junk = junk
                        if half == 1:
                            nc.gpsimd.tensor_copy(
                                acc[:, c : c + 1], junk[:, HBANK - 1 : HBANK]
                            )
                    elif MODE == "ttr":
                        s1 = copies.tile([P, HBANK], f32, tag="s1")
                        nc.scalar.copy(s1[:], p1[:])
                        junk = scratch.tile([P, HBANK], f32, tag="junk")
                        col_i = 2 * c + half
                        nc.vector.tensor_tensor_reduce(
                            out=junk[:],
                            in0=p0[:],
                            in1=s1[:],
                            scale=1.0,
                            scalar=BIG,
                            op0=MIN,
                            op1=MIN,
                            accum_out=acc[:, col_i : col_i + 1],
                        )
                    else:
                        col_i = 4 * c + 2 * half
                        nc.vector.tensor_reduce(
                            acc[:, col_i : col_i + 1], p0[:], axis=AX, op=MIN
                        )
                        nc.vector.tensor_reduce(
                            acc[:, col_i + 1 : col_i + 2], p1[:], axis=AX, op=MIN
                        )
            # per-point min of the group columns, then per-partition sum
            if cols_per_chunk == 1:
                rm = acc
            else:
                rm = singles.tile([P, NCH], f32, tag=f"rm_{li}")
                nc.vector.tensor_reduce(
                    rm[:],
                    acc[:].rearrange("p (c h) -> p c h", h=cols_per_chunk),
                    axis=AX,
                    op=MIN,
                )
            nc.vector.tensor_reduce(rs_all[:, li : li + 1], rm[:], axis=AX, op=ADD)
        nc.sync.dma_start(out=res_d[:, :], in_=rs_all[:])

    if not nc.is_finalized():
        nc.finalize()
    return nc


def _get_nc():
    if "nc" not in _CACHE:
        _CACHE["nc"] = _build_nc()
    return _CACHE["nc"]


def _postprocess(results):
    out = np.empty(B, np.float32)
    for b in range(B):
        r = np.asarray(results[b]["res"], dtype=np.float64)  # [128, 2]
        out[b] = (r[:, 0].sum() + r[:, 1].sum()) / N
    return out


def kernel(x, y):
    from concourse.bass_utils import run_bass_kernel_spmd

    x = np.ascontiguousarray(np.asarray(x, dtype=np.float32))
    y = np.ascontiguousarray(np.asarray(y, dtype=np.float32))
    assert x.shape == (B, N, D) and y.shape == (B, N, D)
    nc = _get_nc()
    in_maps = [{"x": x[b], "y": y[b]} for b in range(B)]
    res = run_bass_kernel_spmd(nc, in_maps, core_ids=list(range(B)))
    return _postprocess(res.results)


def timed_run(x, y, **kwargs):
    """Run with NTFF tracing; returns (output, exec_time_ns)."""
    from concourse.bass_utils import run_bass_kernel_spmd

    x = np.ascontiguousarray(np.asarray(x, dtype=np.float32))
    y = np.ascontiguousarray(np.asarray(y, dtype=np.float32))
    nc = _get_nc()
    in_maps = [{"x": x[b], "y": y[b]} for b in range(B)]
    res = run_bass_kernel_spmd(
        nc, in_maps, core_ids=list(range(B)), trace=True, **kwargs
    )
    return _postprocess(res.results), res.exec_time_ns


# revision 35
# speedup vs baseline: 1.0096x; 1.0096x over previous
"""Chamfer distance kernel for Trainium2 (8 NeuronCores, data-parallel over batch).

Full inputs x, y: [8, 4096, 3] fp32. Output: [8] fp32.

Strategy (per core = one batch):
  dist(i,j) = ||x_i||^2 + ||y_j||^2 - 2 x_i.y_j  computed on the PE as a
  K=24 bf16 matmul using 3-way bf16 splits of coordinates and norms
  (fp32-equivalent: ~1e-5 abs error on distances).  Both min directions
  run the same pipeline with the two matmul layouts (x-on-partitions and
  y-on-partitions, swapping lhsT/rhs roles of the same two K-matrices).
  Each [128, 4096] row-block lands in PSUM as four 2-bank groups and is
  consumed by:
    - ScalarE copies of the odd groups PSUM->SBUF (parallel bank access)
    - VectorE tensor_tensor_scan(min, min): one op consumes a PSUM group
      plus a copied SBUF group; the scan's last column (chained via
      `initial`) is the per-point running min -> per-chunk min column.
  GpSimd extracts the scan tails; VectorE reduces columns to per-partition
  row-sums; host sums 128 values per direction and divides by n.
"""

import os
import sys

import numpy as np

for _p in ("/opt/trn_rl_repo", "/root/.axon_site/_ro/trn_rl_repo"):
    if os.path.isdir(_p) and _p not in sys.path:
        sys.path.insert(0, _p)

B = 8
N = 4096
D = 3
P = 128
IPP = N // P  # 32 points per partition
K = 24        # contraction rows
NCH = N // P  # 32 lhsT chunks of 128 points
HBANK = 1024  # free elems per 2-bank psum group
BIG = 3.0e38
MODE = os.environ.get("CHAMFER_MODE", "tts")  # "tts" | "tts2k" | "reduce" | "ttr"

_CACHE = {}


def _build_nc():
    from contextlib import ExitStack

    from concourse import bacc, mybir
    from concourse.tile import TileContext

    f32 = mybir.dt.float32
    bf16 = mybir.dt.bfloat16
    MIN = mybir.AluOpType.min
    ADD = mybir.AluOpType.add
    AX = mybir.AxisListType.X

    nc = bacc.Bacc()
    x_d = nc.declare_dram_parameter("x", [N, D], f32, isOutput=False)
    y_d = nc.declare_dram_parameter("y", [N, D], f32, isOutput=False)
    res_d = nc.declare_dram_parameter("res", [P, 2], f32, isOutput=True)

    with ExitStack() as ctx:
        tc = ctx.enter_context(TileContext(nc))
        singles = ctx.enter_context(tc.tile_pool(name="singles", bufs=1))
        copies = ctx.enter_context(tc.tile_pool(name="copies", bufs=3))
        scratch = ctx.enter_context(tc.tile_pool(name="scratch", bufs=2))

        # ---------- operand prep: K-matrices KM[side] of shape [24, 4096] bf16
        # Row pairing (lhs row r multiplies rhs row r, summed over r):
        #   r0-2 : 2*xh_c * -yh_c     r3-5 : 2*xh_c * -ym_c
        #   r6-8 : 2*xm_c * -yh_c    r9-11: 2*xm_c * -ym_c
        #   r12-14: 2*xh_c * -yl_c    r15-17: 2*xl_c * -yh_c
        #   r18-20: nx{h,m,l} * 1     r21-23: 1 * ny{h,m,l}
        # Built by assembling W[side] = [128 pts, 32 blocks, 32 rows] bf16 and
        # PE-transposing each [128, 32] block into KM[24, block*128:...].
        from concourse import masks

        ident = singles.tile([P, P], bf16, tag="ident")
        masks.make_identity(nc, ident[:])

        KM = {}
        WS = {}
        for side, dram in (("y", y_d), ("x", x_d)):
            sc = 2.0 if side == "x" else -1.0
            raw = singles.tile([P, IPP, D], f32, tag=f"raw_{side}")
            nc.sync.dma_start(
                out=raw[:],
                in_=dram[:, :].rearrange("(p i) c -> p i c", p=P),
            )
            # 3-way bf16 split of coordinates (natural layout).
            # Casts/subs go to ScalarE/GpSimd to keep VectorE (the main-loop
            # bottleneck engine) free.
            h = singles.tile([P, IPP, D], bf16, tag=f"h_{side}")
            nc.scalar.copy(h[:], raw[:])
            e1 = singles.tile([P, IPP, D], f32, tag=f"e1_{side}")
            nc.gpsimd.tensor_sub(e1[:], raw[:], h[:])
            m = singles.tile([P, IPP, D], bf16, tag=f"m_{side}")
            nc.scalar.copy(m[:], e1[:])
            e2 = singles.tile([P, IPP, D], f32, tag=f"e2_{side}")
            nc.gpsimd.tensor_sub(e2[:], e1[:], m[:])
            low = singles.tile([P, IPP, D], bf16, tag=f"l_{side}")
            nc.scalar.copy(low[:], e2[:])
            # norms + 3-way split
            sq = singles.tile([P, IPP, D], f32, tag=f"sq_{side}")
            nc.gpsimd.tensor_mul(sq[:], raw[:], raw[:])
            nrm = singles.tile([P, IPP], f32, tag=f"nrm_{side}")
            nc.vector.tensor_reduce(nrm[:], sq[:], axis=AX, op=ADD)
            nh = singles.tile([P, IPP], bf16, tag=f"nh_{side}")
            nc.scalar.copy(nh[:], nrm[:])
            ne1 = singles.tile([P, IPP], f32, tag=f"ne1_{side}")
            nc.gpsimd.tensor_sub(ne1[:], nrm[:], nh[:])
            nm = singles.tile([P, IPP], bf16, tag=f"nm_{side}")
            nc.scalar.copy(nm[:], ne1[:])
            ne2 = singles.tile([P, IPP], f32, tag=f"ne2_{side}")
            nc.gpsimd.tensor_sub(ne2[:], ne1[:], nm[:])
            nl = singles.tile([P, IPP], bf16, tag=f"nl_{side}")
            nc.scalar.copy(nl[:], ne2[:])

            # staging tile W: [128 pts-in-block, 32 blocks, 32 rows] bf16
            w = singles.tile([P, IPP, 32], bf16, tag=f"w_{side}")
            nc.gpsimd.memset(w[:], 1.0)  # rows 21-23 / 18-20 stay ones; 24-31 pad
            if side == "x":
                rowsrc = [(h, sc), (h, sc), (m, sc), (m, sc), (h, sc), (low, sc)]
                norm0 = 18
            else:
                rowsrc = [(h, sc), (m, sc), (h, sc), (m, sc), (low, sc), (h, sc)]
                norm0 = 21
            # y-side assembly on VectorE (idle during startup, gates main
            # start); x-side on GpSimd (VectorE is busy once main runs).
            asm = nc.vector if side == "y" else nc.gpsimd
            for g, (arr, s) in enumerate(rowsrc):
                for c in range(D):
                    r = 3 * g + c
                    asm.tensor_scalar_mul(w[:, :, r], arr[:, :, c], s)
            for c, arr in enumerate((nh, nm, nl)):
                asm.tensor_copy(w[:, :, norm0 + c], arr[:])

            km = singles.tile([32, N], bf16, tag=f"km_{side}")
            KM[side] = km
            WS[side] = w

        # ---------- main: two layouts, each 32 chunks x 2 half-rows
        psum_bufs = 1 if MODE == "tts2k" else 2
        psum = ctx.enter_context(
            tc.tile_pool(name="psum", bufs=psum_bufs, space="PSUM")
        )

        def emit_km_block(side, t4):
            # transpose 4 blocks into one PSUM bank, then copy out [32, 512]
            w = WS[side]
            km = KM[side]
            borrow = "pa" if MODE == "tts2k" else "p1"
            pt = psum.tile([32, 512], bf16, tag=borrow)  # borrow main psum slots
            for u in range(4):
                t = t4 * 4 + u
                nc.tensor.transpose(
                    pt[:, u * P : (u + 1) * P], w[:, t, :], ident[:]
                )
            if side == "y":
                nc.vector.tensor_copy(km[:, t4 * 512 : (t4 + 1) * 512], pt[:])
            else:
                nc.scalar.copy(km[:, t4 * 512 : (t4 + 1) * 512], pt[:])

        # y-side KM needed in full before the first matmul; x-side KM blocks
        # are emitted just-in-time inside the layout-A chunk loop.
        for t4 in range(IPP // 4):
            emit_km_block("y", t4)
        rs_all = singles.tile([P, 2], f32, tag="rs_all")
        cols_per_chunk = {"tts": 1, "tts2k": 1, "ttr": 2, "reduce": 4}[MODE]
        for li, (lhs_km, rhs_km) in enumerate(
            [(KM["x"], KM["y"]), (KM["y"], KM["x"])]
        ):
            acc = singles.tile([P, cols_per_chunk * NCH], f32, tag=f"acc_{li}")
            for c in range(NCH):
                if li == 0 and c % 4 == 0:
                    emit_km_block("x", c // 4)
                lhsT = lhs_km[0:K, c * P : (c + 1) * P]
                if MODE == "tts2k":
                    # FD-2048 groups: one ScalarE copy + one scan per chunk
                    pa = psum.tile([P, 2 * HBANK], f32, tag="pa")
                    pb = psum.tile([P, 2 * HBANK], f32, tag="pb")
                    for j in range(8):
                        dst = pa if j < 4 else pb
                        col = (j % 4) * 512
                        nc.tensor.matmul(
                            dst[:, col : col + 512],
                            lhsT,
                            rhs_km[0:K, j * 512 : (j + 1) * 512],
                            start=True,
                            stop=True,
                        )
                    s1 = copies.tile([P, 2 * HBANK], f32, tag="s1")
                    nc.scalar.copy(s1[:], pa[:])
                    junk = scratch.tile([P, 2 * HBANK], f32, tag="junk")
                    nc.vector.tensor_tensor_scan(
                        out=junk[:],
                        data0=pb[:],
                        data1=s1[:],
                        initial=BIG,
                        op0=MIN,
                        op1=MIN,
                    )
                    nc.gpsimd.tensor_copy(
                        acc[:, c : c + 1], junk[:, 2 * HBANK - 1 : 2 * HBANK]
                    )
                    continue
                prev_junk = None
                for half in range(2):
                    p0 = psum.tile([P, HBANK], f32, tag="p0")
                    p1 = psum.tile([P, HBANK], f32, tag="p1")
                    for q in range(4):
                        j = half * 4 + q
                        dst = p0 if q < 2 else p1
                        col = (q % 2) * 512
                        nc.tensor.matmul(
                            dst[:, col : col + 512],
                            lhsT,
                            rhs_km[0:K, j * 512 : (j + 1) * 512],
                            start=True,
                            stop=True,
                        )
                    if MODE == "tts":
                        s1 = copies.tile([P, HBANK], f32, tag="s1")
                        nc.scalar.copy(s1[:], p1[:])
                        junk = scratch.tile([P, HBANK], f32, tag="junk")
                        nc.vector.tensor_tensor_scan(
                            out=junk[:],
                            data0=p0[:],
                            data1=s1[:],
                            initial=(
                                BIG if prev_junk is None
                                else prev_junk[:, HBANK - 1 : HBANK]
                            ),
                            op0=MIN,
                            op1=MIN,
                        )
                        prev_junk = junk
                        if half == 1:
                            nc.gpsimd.tensor_copy(
                                acc[:, c : c + 1], junk[:, HBANK - 1 : HBANK]
                            )
                    elif MODE == "ttr":
                        s1 = copies.tile([P, HBANK], f32, tag="s1")
                        nc.scalar.copy(s1[:], p1[:])
                        junk = scratch.tile([P, HBANK], f32, tag="junk")
                        col_i = 2 * c + half
                        nc.vector.tensor_tensor_reduce(
                            out=junk[:],
                            in0=p0[:],
                            in1=s1[:],
                            scale=1.0,
                            scalar=BIG,
                            op0=MIN,
                            op1=MIN,
                            accum_out=acc[:, col_i : col_i + 1],
                        )
                    else:
                        col_i = 4 * c + 2 * half
                        nc.vector.tensor_reduce(
                            acc[:, col_i : col_i + 1], p0[:], axis=AX, op=MIN
                        )
                        nc.vector.tensor_reduce(
                            acc[:, col_i + 1 : col_i + 2], p1[:], axis=AX, op=MIN
                        )
            # per-point min of the group columns, then per-partition sum
            if cols_per_chunk == 1:
                rm = acc
            else:
                rm = singles.tile([P, NCH], f32, tag=f"rm_{li}")
                nc.vector.tensor_reduce(
                    rm[:],
                    acc[:].rearrange("p (c h) -> p c h", h=cols_per_chunk),
                    axis=AX,
                    op=MIN,
                )
            nc.vector.tensor_reduce(rs_all[:, li : li + 1], rm[:], axis=AX, op=ADD)
        nc.sync.dma_start(out=res_d[:, :], in_=rs_all[:])

    if not nc.is_finalized():
        nc.finalize()
    return nc


def _get_nc():
    if "nc" not in _CACHE:
        _CACHE["nc"] = _build_nc()
    return _CACHE["nc"]


def _postprocess(results):
    out = np.empty(B, np.float32)
    for b in range(B):
        r = np.asarray(results[b]["res"], dtype=np.float64)  # [128, 2]
        out[b] = (r[:, 0].sum() + r[:, 1].sum()) / N
    return out


def kernel(x, y):
    from concourse.bass_utils import run_bass_kernel_spmd

    x = np.ascontiguousarray(np.asarray(x, dtype=np.float32))
    y = np.ascontiguousarray(np.asarray(y, dtype=np.float32))
    assert x.shape == (B, N, D) and y.shape == (B, N, D)
    nc = _get_nc()
    in_maps = [{"x": x[b], "y": y[b]} for b in range(B)]
    res = run_bass_kernel_spmd(nc, in_maps, core_ids=list(range(B)))
    return _postprocess(res.results)


def timed_run(x, y, **kwargs):
    """Run with NTFF tracing; returns (output, exec_time_ns)."""
    from concourse.bass_utils import run_bass_kernel_spmd

    x = np.ascontiguousarray(np.asarray(x, dtype=np.float32))
    y = np.ascontiguousarray(np.asarray(y, dtype=np.float32))
    nc = _get_nc()
    in_maps = [{"x": x[b], "y": y[b]} for b in range(B)]
    res = run_bass_kernel_spmd(
        nc, in_maps, core_ids=list(range(B)), trace=True, **kwargs
    )
    return _postprocess(res.results), res.exec_time_ns


# revision 41
# speedup vs baseline: 1.0100x; 1.0003x over previous
"""Chamfer distance kernel for Trainium2 (8 NeuronCores, data-parallel over batch).

Full inputs x, y: [8, 4096, 3] fp32. Output: [8] fp32.

Strategy (per core = one batch):
  dist(i,j) = ||x_i||^2 + ||y_j||^2 - 2 x_i.y_j  computed on the PE as a
  K=24 bf16 matmul using 3-way bf16 splits of coordinates and norms
  (fp32-equivalent: ~1e-5 abs error on distances).  Both min directions
  run the same pipeline with the two matmul layouts (x-on-partitions and
  y-on-partitions, swapping lhsT/rhs roles of the same two K-matrices).
  Each [128, 4096] row-block lands in PSUM as four 2-bank groups and is
  consumed by:
    - ScalarE copies of the odd groups PSUM->SBUF (parallel bank access)
    - VectorE tensor_tensor_scan(min, min): one op consumes a PSUM group
      plus a copied SBUF group; the scan's last column (chained via
      `initial`) is the per-point running min -> per-chunk min column.
  GpSimd extracts the scan tails; VectorE reduces columns to per-partition
  row-sums; host sums 128 values per direction and divides by n.
"""

import os
import sys

import numpy as np

for _p in ("/opt/trn_rl_repo", "/root/.axon_site/_ro/trn_rl_repo"):
    if os.path.isdir(_p) and _p not in sys.path:
        sys.path.insert(0, _p)

B = 8
N = 4096
D = 3
P = 128
IPP = N // P  # 32 points per partition
K = 24        # contraction rows
NCH = N // P  # 32 lhsT chunks of 128 points
HBANK = 1024  # free elems per 2-bank psum group
BIG = 3.0e38
MODE = os.environ.get("CHAMFER_MODE", "tts")  # "tts" | "tts2k" | "reduce" | "ttr"

_CACHE = {}


def _build_nc():
    from contextlib import ExitStack

    from concourse import bacc, mybir
    from concourse.tile import TileContext

    f32 = mybir.dt.float32
    bf16 = mybir.dt.bfloat16
    MIN = mybir.AluOpType.min
    ADD = mybir.AluOpType.add
    AX = mybir.AxisListType.X

    nc = bacc.Bacc()
    x_d = nc.declare_dram_parameter("x", [N, D], f32, isOutput=False)
    y_d = nc.declare_dram_parameter("y", [N, D], f32, isOutput=False)
    res_d = nc.declare_dram_parameter("res", [P, 2], f32, isOutput=True)

    with ExitStack() as ctx:
        tc = ctx.enter_context(TileContext(nc))
        singles = ctx.enter_context(tc.tile_pool(name="singles", bufs=1))
        copies = ctx.enter_context(tc.tile_pool(name="copies", bufs=3))
        scratch = ctx.enter_context(tc.tile_pool(name="scratch", bufs=2))

        # ---------- operand prep: K-matrices KM[side] of shape [24, 4096] bf16
        # Row pairing (lhs row r multiplies rhs row r, summed over r):
        #   r0-2 : 2*xh_c * -yh_c     r3-5 : 2*xh_c * -ym_c
        #   r6-8 : 2*xm_c * -yh_c    r9-11: 2*xm_c * -ym_c
        #   r12-14: 2*xh_c * -yl_c    r15-17: 2*xl_c * -yh_c
        #   r18-20: nx{h,m,l} * 1     r21-23: 1 * ny{h,m,l}
        # Built by assembling W[side] = [128 pts, 32 blocks, 32 rows] bf16 and
        # PE-transposing each [128, 32] block into KM[24, block*128:...].
        from concourse import masks

        # Warmup: dependency-free first ops per engine so one-time costs
        # (ACT table load ~2.7us, GpSimd library load) overlap the input DMAs
        # instead of sitting in the splits dependency chain.
        warm = singles.tile([P, 8], f32, tag="warm")
        nc.vector.memset(warm[:, 0:4], 0.0)
        nc.scalar.copy(warm[:, 4:6], warm[:, 0:2])
        nc.gpsimd.tensor_copy(warm[:, 6:8], warm[:, 2:4])

        ident = singles.tile([P, P], bf16, tag="ident")
        masks.make_identity(nc, ident[:])

        KM = {}
        WS = {}
        for side, dram in (("y", y_d), ("x", x_d)):
            sc = 2.0 if side == "x" else -1.0
            raw = singles.tile([P, IPP, D], f32, tag=f"raw_{side}")
            nc.sync.dma_start(
                out=raw[:],
                in_=dram[:, :].rearrange("(p i) c -> p i c", p=P),
            )
            # 3-way bf16 split of coordinates (natural layout).
            # Casts/subs go to ScalarE/GpSimd to keep VectorE (the main-loop
            # bottleneck engine) free.
            h = singles.tile([P, IPP, D], bf16, tag=f"h_{side}")
            nc.scalar.copy(h[:], raw[:])
            e1 = singles.tile([P, IPP, D], f32, tag=f"e1_{side}")
            sub_eng = nc.vector if side == "y" else nc.gpsimd
            sub_eng.tensor_sub(e1[:], raw[:], h[:])
            m = singles.tile([P, IPP, D], bf16, tag=f"m_{side}")
            nc.scalar.copy(m[:], e1[:])
            e2 = singles.tile([P, IPP, D], f32, tag=f"e2_{side}")
            sub_eng.tensor_sub(e2[:], e1[:], m[:])
            low = singles.tile([P, IPP, D], bf16, tag=f"l_{side}")
            nc.scalar.copy(low[:], e2[:])
            # norms + 3-way split
            sq = singles.tile([P, IPP, D], f32, tag=f"sq_{side}")
            nc.gpsimd.tensor_mul(sq[:], raw[:], raw[:])
            nrm = singles.tile([P, IPP], f32, tag=f"nrm_{side}")
            nc.vector.tensor_reduce(nrm[:], sq[:], axis=AX, op=ADD)
            nh = singles.tile([P, IPP], bf16, tag=f"nh_{side}")
            nc.scalar.copy(nh[:], nrm[:])
            ne1 = singles.tile([P, IPP], f32, tag=f"ne1_{side}")
            nc.gpsimd.tensor_sub(ne1[:], nrm[:], nh[:])
            nm = singles.tile([P, IPP], bf16, tag=f"nm_{side}")
            nc.scalar.copy(nm[:], ne1[:])
            ne2 = singles.tile([P, IPP], f32, tag=f"ne2_{side}")
            nc.gpsimd.tensor_sub(ne2[:], ne1[:], nm[:])
            nl = singles.tile([P, IPP], bf16, tag=f"nl_{side}")
            nc.scalar.copy(nl[:], ne2[:])

            # staging tile W: [128 pts-in-block, 32 blocks, 32 rows] bf16
            w = singles.tile([P, IPP, 32], bf16, tag=f"w_{side}")
            nc.gpsimd.memset(w[:], 1.0)  # rows 21-23 / 18-20 stay ones; 24-31 pad
            if side == "x":
                rowsrc = [(h, sc), (h, sc), (m, sc), (m, sc), (h, sc), (low, sc)]
                norm0 = 18
            else:
                rowsrc = [(h, sc), (m, sc), (h, sc), (m, sc), (low, sc), (h, sc)]
                norm0 = 21
            # y-side assembly on VectorE (idle during startup, gates main
            # start); x-side on GpSimd (VectorE is busy once main runs).
            asm = nc.vector if side == "y" else nc.gpsimd
            for g, (arr, s) in enumerate(rowsrc):
                for c in range(D):
                    r = 3 * g + c
                    asm.tensor_scalar_mul(w[:, :, r], arr[:, :, c], s)
            for c, arr in enumerate((nh, nm, nl)):
                asm.tensor_copy(w[:, :, norm0 + c], arr[:])

            km = singles.tile([32, N], bf16, tag=f"km_{side}")
            KM[side] = km
            WS[side] = w

        # ---------- main: two layouts, each 32 chunks x 2 half-rows
        psum_bufs = 1 if MODE == "tts2k" else 2
        psum = ctx.enter_context(
            tc.tile_pool(name="psum", bufs=psum_bufs, space="PSUM")
        )

        def emit_km_block(side, t4):
            # transpose 4 blocks into one PSUM bank, then copy out [32, 512]
            w = WS[side]
            km = KM[side]
            borrow = "pa" if MODE == "tts2k" else "p1"
            pt = psum.tile([32, 512], bf16, tag=borrow)  # borrow main psum slots
            for u in range(4):
                t = t4 * 4 + u
                nc.tensor.transpose(
                    pt[:, u * P : (u + 1) * P], w[:, t, :], ident[:]
                )
            if side == "y":
                nc.vector.tensor_copy(km[:, t4 * 512 : (t4 + 1) * 512], pt[:])
            else:
                nc.scalar.copy(km[:, t4 * 512 : (t4 + 1) * 512], pt[:])

        # y-side KM needed in full before the first matmul; x-side KM blocks
        # are emitted just-in-time inside the layout-A chunk loop.
        for t4 in range(IPP // 4):
            emit_km_block("y", t4)
        rs_all = singles.tile([P, 2], f32, tag="rs_all")
        cols_per_chunk = {"tts": 1, "tts2k": 1, "ttr": 2, "reduce": 4}[MODE]
        for li, (lhs_km, rhs_km) in enumerate(
            [(KM["x"], KM["y"]), (KM["y"], KM["x"])]
        ):
            acc = singles.tile([P, cols_per_chunk * NCH], f32, tag=f"acc_{li}")
            for c in range(NCH):
                if li == 0 and c % 4 == 0:
                    emit_km_block("x", c // 4)
                lhsT = lhs_km[0:K, c * P : (c + 1) * P]
                if MODE == "tts2k":
                    # FD-2048 groups: one ScalarE copy + one scan per chunk
                    pa = psum.tile([P, 2 * HBANK], f32, tag="pa")
                    pb = psum.tile([P, 2 * HBANK], f32, tag="pb")
                    for j in range(8):
                        dst = pa if j < 4 else pb
                        col = (j % 4) * 512
                        nc.tensor.matmul(
                            dst[:, col : col + 512],
                            lhsT,
                            rhs_km[0:K, j * 512 : (j + 1) * 512],
                            start=True,
                            stop=True,
                        )
                    s1 = copies.tile([P, 2 * HBANK], f32, tag="s1")
                    nc.scalar.copy(s1[:], pa[:])
                    junk = scratch.tile([P, 2 * HBANK], f32, tag="junk")
                    nc.vector.tensor_tensor_scan(
                        out=junk[:],
                        data0=pb[:],
                        data1=s1[:],
                        initial=BIG,
                        op0=MIN,
                        op1=MIN,
                    )
                    nc.gpsimd.tensor_copy(
                        acc[:, c : c + 1], junk[:, 2 * HBANK - 1 : 2 * HBANK]
                    )
                    continue
                prev_junk = None
                for half in range(2):
                    p0 = psum.tile([P, HBANK], f32, tag="p0")
                    p1 = psum.tile([P, HBANK], f32, tag="p1")
                    for q in range(4):
                        j = half * 4 + q
                        dst = p0 if q < 2 else p1
                        col = (q % 2) * 512
                        nc.tensor.matmul(
                            dst[:, col : col + 512],
                            lhsT,
                            rhs_km[0:K, j * 512 : (j + 1) * 512],
                            start=True,
                            stop=True,
                        )
                    if MODE == "tts":
                        s1 = copies.tile([P, HBANK], f32, tag="s1")
                        nc.scalar.copy(s1[:], p1[:])
                        junk = scratch.tile([P, HBANK], f32, tag="junk")
                        nc.vector.tensor_tensor_scan(
                            out=junk[:],
                            data0=p0[:],
                            data1=s1[:],
                            initial=(
                                BIG if prev_junk is None
                                else prev_junk[:, HBANK - 1 : HBANK]
                            ),
                            op0=MIN,
                            op1=MIN,
                        )
                        prev_junk = junk
                        if half == 1:
                            nc.gpsimd.tensor_copy(
                                acc[:, c : c + 1], junk[:, HBANK - 1 : HBANK]
                            )
                    elif MODE == "ttr":
                        s1 = copies.tile([P, HBANK], f32, tag="s1")
                        nc.scalar.copy(s1[:], p1[:])
                        junk = scratch.tile([P, HBANK], f32, tag="junk")
                        col_i = 2 * c + half
                        nc.vector.tensor_tensor_reduce(
                            out=junk[:],
                            in0=p0[:],
                            in1=s1[:],
                            scale=1.0,
                            scalar=BIG,
                            op0=MIN,
                            op1=MIN,
                            accum_out=acc[:, col_i : col_i + 1],
                        )
                    else:
                        col_i = 4 * c + 2 * half
                        nc.vector.tensor_reduce(
                            acc[:, col_i : col_i + 1], p0[:], axis=AX, op=MIN
                        )
                        nc.vector.tensor_reduce(
                            acc[:, col_i + 1 : col_i + 2], p1[:], axis=AX, op=MIN
                        )
            # per-point min of the group columns, then per-partition sum
            if cols_per_chunk == 1:
                rm = acc
            else:
                rm = singles.tile([P, NCH], f32, tag=f"rm_{li}")
                nc.vector.tensor_reduce(
                    rm[:],
                    acc[:].rearrange("p (c h) -> p c h", h=cols_per_chunk),
                    axis=AX,
                    op=MIN,
                )
            nc.vector.tensor_reduce(rs_all[:, li : li + 1], rm[:], axis=AX, op=ADD)
        nc.sync.dma_start(out=res_d[:, :], in_=rs_all[:])

    if not nc.is_finalized():
        nc.finalize()
    return nc


def _get_nc():
    if "nc" not in _CACHE:
        _CACHE["nc"] = _build_nc()
    return _CACHE["nc"]


def _postprocess(results):
    out = np.empty(B, np.float32)
    for b in range(B):
        r = np.asarray(results[b]["res"], dtype=np.float64)  # [128, 2]
        out[b] = (r[:, 0].sum() + r[:, 1].sum()) / N
    return out


def kernel(x, y):
    from concourse.bass_utils import run_bass_kernel_spmd

    x = np.ascontiguousarray(np.asarray(x, dtype=np.float32))
    y = np.ascontiguousarray(np.asarray(y, dtype=np.float32))
    assert x.shape == (B, N, D) and y.shape == (B, N, D)
    nc = _get_nc()
    in_maps = [{"x": x[b], "y": y[b]} for b in range(B)]
    res = run_bass_kernel_spmd(nc, in_maps, core_ids=list(range(B)))
    return _postprocess(res.results)


def timed_run(x, y, **kwargs):
    """Run with NTFF tracing; returns (output, exec_time_ns)."""
    from concourse.bass_utils import run_bass_kernel_spmd

    x = np.ascontiguousarray(np.asarray(x, dtype=np.float32))
    y = np.ascontiguousarray(np.asarray(y, dtype=np.float32))
    nc = _get_nc()
    in_maps = [{"x": x[b], "y": y[b]} for b in range(B)]
    res = run_bass_kernel_spmd(
        nc, in_maps, core_ids=list(range(B)), trace=True, **kwargs
    )
    return _postprocess(res.results), res.exec_time_ns
